# revision 49
# baseline (speedup 1.0000x reference)
"""Multi-head attention (B=2, S=2048, E=2048, H=16) on 8 Trainium2 NeuronCores.

Sharding: core c = 4*b + g handles batch b and head group g (4 heads).
Each core computes QKV projections for its heads, attention, and its partial
output projection; the host sums the 4 partials per batch and adds the
(folded) biases.

Bias folding (host side):
  - q bias: applied on-chip (per-partition bias in the qT projection copy).
  - k bias: dropped — adds a per-query constant to scores, softmax-invariant.
  - v bias: since softmax weights sum to 1, it shifts attn by bv exactly;
    folded into the final bias as bv_cat @ W_out.
  - out bias: added on host after summing partials.

Schedule (single pass, engines overlapped):
  window 1: V projection pipelined against the serial DMA pipe (groups of 8
    sequence tiles accumulate in 8 PSUM banks while xT/wv k-tiles stream in),
    then Q/K projection for all but the last tile; the last q/k tile is
    deferred into the first attention cycle as PE filler.
  window 2: attention emitted sb-major (4 heads per sb block); out-projection
    chunks of the previous sb are interleaved between attention matmuls so
    the Act engine's exp stream (the secondary bottleneck) hides under PE
    work.  Softmax denominator = DVE usum adds + fp16 ones-matmul; the
    epilogue (den/recip/broadcast/normalize) is deferred into the next
    block so the tensor engine never waits on the DVE stream.
"""

import os
import shutil
import hashlib
from collections import deque

import numpy as np
import ml_dtypes

import concourse.bass as bass
import concourse.mybir as mybir
import concourse.tile as tile

# ---------------------------------------------------------------------------
# problem constants (hardcoded per contest rules)
B, S, E, H, DH = 2, 2048, 2048, 16, 128
NCORES = 8
HPC = H // (NCORES // B)  # heads per core = 4

F32 = mybir.dt.float32


_GATE_SEM_ID = 170  # Tile uses ids ~151-165; this one is free


def _split_waits(nc, limit=1):
    """Cap sync-waits per instruction (walrus here rejects more than one).

    Compute-engine instructions block their sequencer anyway, so surplus
    waits move onto same-engine nops inserted immediately before them —
    semantics unchanged.

    DMA instructions are dispatched asynchronously to DGE queues; making
    the issuing sequencer block on their waits can deadlock. Instead, a
    chain of Pool-engine nops takes over ALL of the DMA's waits and finally
    increments a dedicated gate semaphore; the DMA waits only on the gate.
    The gate semaphore is returned to zero at the very end so repeated NEFF
    executions see a clean state.
    """
    import bass_rust

    counter = [0]
    gate_count = [0]

    def mk_nop(engine, hint, waits, updates):
        counter[0] += 1
        nop = mybir.InstNoOp(
            name=f"W-split-{counter[0]}", opcode="NoOp",
            engine=engine, ins=[], outs=[],
            text_hint=hint, bass_nofuse=True,
        )
        nop.sync_info = bass_rust.SyncInfo(on_wait=waits, on_update=updates)
        nc.register_instruction(nop, overwrite=True)
        return nop

    last_blk = None
    for fn in nc.m.functions:
        for blk in fn.blocks:
            old = list(blk.instructions)
            if old:
                last_blk = blk
            new = []
            changed = False
            for inst in old:
                si = inst.sync_info
                n = len(si.on_wait) if si is not None else 0
                if n > limit:
                    waits = list(si.on_wait)
                    changed = True
                    if "DMA" in type(inst).__name__:
                        gate_count[0] += 1
                        for j, w in enumerate(waits):
                            upd = []
                            if j == len(waits) - 1:
                                upd = [bass_rust.SyncUpdate(
                                    sync_type="semaphore", id=_GATE_SEM_ID,
                                    update_mode="sem-inc", update_value=1,
                                    ant_name="dma_gate")]
                            new.append(mk_nop(
                                mybir.EngineType.Pool, "dma_gate", [w], upd))
                        si.on_wait = [bass_rust.SyncWait(
                            sync_type="semaphore", id=_GATE_SEM_ID,
                            wait_mode="sem-ge-imm",
                            wait_value=gate_count[0], ant_name="dma_gate")]
                        inst.sync_info = si
                    else:
                        for j in range(limit, n, limit):
                            new.append(mk_nop(
                                inst.engine, "wait_split",
                                waits[j:j + limit], []))
                        si.on_wait = waits[:limit]
                        inst.sync_info = si
                new.append(inst)
            if changed:
                blk.instructions = new

    if gate_count[0] and last_blk is not None:
        reset = mk_nop(mybir.EngineType.Pool, "gate_reset", [], [
            bass_rust.SyncUpdate(
                sync_type="semaphore", id=_GATE_SEM_ID,
                update_mode="sem-sub-imm", update_value=gate_count[0],
                ant_name="dma_gate")])
        lst = list(last_blk.instructions)
        lst.append(reset)
        last_blk.instructions = lst


def _install_neff_cache():
    """Persistently cache walrus-compiled NEFFs keyed on the BIR hash."""
    import concourse.bass2jax as b2j
    import concourse.bass_utils as bu

    if getattr(b2j, "_ant_neff_cache_installed", False):
        return
    cache_dir = os.path.expanduser("~/.cache/bass_neff_cache")
    os.makedirs(cache_dir, exist_ok=True)
    orig = bu.compile_bir_kernel

    def cached_compile(bir_json, tmpdir, neff_name="file.neff"):
        if isinstance(bir_json, str):
            bir_bytes = bir_json.encode()
        else:
            bir_bytes = bir_json
        h = hashlib.sha256(bir_bytes).hexdigest()
        path = os.path.join(cache_dir, f"{h}_{neff_name}")
        if os.path.exists(path):
            os.makedirs(os.path.join(tmpdir, "sg00"), exist_ok=True)
            out = os.path.join(tmpdir, "sg00", neff_name)
            shutil.copy(path, out)
            return out
        out = orig(bir_json, tmpdir, neff_name)
        shutil.copy(out, path)
        return out

    b2j.compile_bir_kernel = cached_compile
    bu.compile_bir_kernel = cached_compile
    b2j._ant_neff_cache_installed = True


# ---------------------------------------------------------------------------
# kernel builder (dims parameterized so a downsized version can run in CoreSim)


def build_nc(s=S, e=E, hpc=HPC, mm_dtype=mybir.dt.float16, split_waits=True,
             vgroup=8, qk_bufs=2, sc_bufs=3, at_bufs=2, op_bufs=2,
             u_bufs=8, usum_bufs=2, rb_bufs=2, stage_bufs=3,
             at_sb_bufs=2):
    """Build the per-core Bass program.

    DRAM inputs (per core):
      xT  : (e, s)        x[b].T
      wqk : (e, 2*hpc*DH) [q_h0 | k_h0 | q_h1 | k_h1 | ...], q pre-scaled
      bq  : (hpc, DH, 1)  q biases (pre-scaled)
      wv  : (e, hpc*DH)   [v_h0 | v_h1 | ...]
      wo  : (hpc*DH, e)   W_out rows for this core's heads
    DRAM output:
      out : (s, e) fp16   partial output (no biases)
    """
    KT = e // 128   # contraction tiles
    ST = s // 128   # sequence tiles
    SB = s // 512   # sequence blocks
    EB = e // 512   # embedding blocks
    STB = ST // SB  # sequence tiles per block (4)
    NQK = 2 * hpc   # q/k projection output tiles
    DT = mm_dtype
    Exp = mybir.ActivationFunctionType.Exp
    Copy = mybir.ActivationFunctionType.Copy
    Ident = mybir.ActivationFunctionType.Identity

    nc = bass.Bass("TRN2", target_bir_lowering=False, debug=False,
                   num_devices=NCORES)
    xT = nc.dram_tensor("xT", [e, s], DT, kind="ExternalInput").ap()
    wqk = nc.dram_tensor("wqk", [e, NQK * DH], DT, kind="ExternalInput").ap()
    bq = nc.dram_tensor("bq", [hpc, DH, 1], F32, kind="ExternalInput").ap()
    wv = nc.dram_tensor("wv", [e, hpc * DH], DT, kind="ExternalInput").ap()
    wo = nc.dram_tensor("wo", [hpc * DH, e], DT, kind="ExternalInput").ap()
    out = nc.dram_tensor("out", [s, e], DT, kind="ExternalOutput").ap()

    with tile.TileContext(nc) as tc:
        singles_cm = tc.tile_pool(name="singles", bufs=1)
        singles = singles_cm.__enter__()
        projx_cm = tc.tile_pool(name="projx", bufs=1)
        projx = projx_cm.__enter__()
        wv_cm = tc.tile_pool(name="wv_pool", bufs=1)
        wvp = wv_cm.__enter__()

        # ---- SBUF tensors
        xT_sb = projx.tile([128, KT, s], DT, tag="xT")
        wqk_sb = projx.tile([128, KT, NQK * DH], DT, tag="wqk")
        wv_sb = wvp.tile([128, KT, hpc * DH], DT, tag="wv")
        wo_sb = singles.tile([128, hpc, e], DT, tag="wo")
        bq_sb = singles.tile([128, hpc, 1], F32, tag="bq")
        qT_sb = singles.tile([128, hpc, s], DT, tag="qT")
        kT_sb = singles.tile([128, hpc, s], DT, tag="kT")
        v_sb = singles.tile([128, ST, hpc * DH], DT, tag="v")
        ones_sb = singles.tile([128, 1], DT, tag="ones")
        ones_row = singles.tile([1, 128], DT, tag="ones_row")
        nc.vector.memset(ones_sb, 1.0)
        nc.vector.memset(ones_row, 1.0)

        # ---- input DMAs: issue order == the serial DMA pipe's service order.
        # v-projection consumes (wv[k], xT[k]) pairs first; wqk afterwards.
        xT_r = xT.rearrange("(ko ki) q -> ki ko q", ki=128)
        wqk_r = wqk.rearrange("(ko ki) q -> ki ko q", ki=128)
        wv_r = wv.rearrange("(ko ki) q -> ki ko q", ki=128)
        wo_r = wo.rearrange("(ho hi) q -> hi ho q", hi=128)
        for k in range(KT):
            if k == 0:
                # xT's first half lands first (the PE warmups below need
                # only it), split so the first real matmul starts sooner
                half = s // 2
                nc.sync.dma_start(out=xT_sb[:, 0, :half],
                                  in_=xT_r[:, 0, :half])
                nc.sync.dma_start(out=wv_sb[:, 0], in_=wv_r[:, 0])
                nc.sync.dma_start(out=xT_sb[:, 0, half:],
                                  in_=xT_r[:, 0, half:])
            else:
                nc.sync.dma_start(out=wv_sb[:, k], in_=wv_r[:, k])
                nc.sync.dma_start(out=xT_sb[:, k], in_=xT_r[:, k])
        for k in range(KT):
            nc.sync.dma_start(out=wqk_sb[:, k], in_=wqk_r[:, k])
        for h in range(hpc):
            nc.sync.dma_start(out=wo_sb[:, h], in_=wo_r[:, h])
            nc.sync.dma_start(out=bq_sb[:, h], in_=bq[h])

        # ---- window 1a: V projection.  The first group runs k-outer so it
        # consumes each (wv[k], xT[k]) pair right as it lands from the DMA
        # pipe.  Later groups run st-outer (inputs are resident by then) so
        # the PSUM drain copies are staggered across the whole group — the
        # next pool's first use waits on ALL of this pool's reads (Tile
        # tracks write-after-read at pool granularity), so the last copy
        # must land right after the last matmul, not 8 serial copies later.
        with tc.tile_pool(name="v_psum", bufs=vgroup, space="PSUM") as vp:
            for g0 in range(0, ST, vgroup):
                n = min(vgroup, ST - g0)
                ps = [vp.tile([128, hpc * DH], F32, tag="v",
                              name=f"vps{g0}_{i}") for i in range(n)]
                if g0 == 0:
                    # tiny warmup matmuls absorb the tensor engine's p-state
                    # ramp while the first DMA lands (they share its gate)
                    for w in range(40):
                        nc.tensor.matmul(
                            ps[0][:32, :32], xT_sb[:, 0, 0:32],
                            xT_sb[:, 0, 0:32], start=True, stop=True,
                            skip_group_check=True,
                        )
                    for k in range(KT):
                        for i in range(n):
                            st = g0 + i
                            nc.tensor.matmul(
                                ps[i], xT_sb[:, k, st * 128:(st + 1) * 128],
                                wv_sb[:, k], start=(k == 0),
                                stop=(k == KT - 1),
                            )
                            if k == KT - 1:
                                nc.scalar.activation(
                                    v_sb[:, g0 + i], ps[i], Copy)
                else:
                    for i in range(n):
                        st = g0 + i
                        for k in range(KT):
                            nc.tensor.matmul(
                                ps[i], xT_sb[:, k, st * 128:(st + 1) * 128],
                                wv_sb[:, k], start=(k == 0),
                                stop=(k == KT - 1),
                            )
                        nc.scalar.activation(v_sb[:, g0 + i], ps[i], Copy)
        wv_cm.__exit__(None, None, None)

        # ---- window 1b: Q/K projections; the last tile (k of the last head)
        # is deferred into the first attention cycle as PE filler.
        # PSUM pool stack (LIFO): sc, atp, dn outlive qk; op replaces qk
        # after the first attention cycle.  3 + 2 + 1 + 2 = 8 banks.
        sc_cm = tc.tile_pool(name="sc_psum", bufs=sc_bufs, space="PSUM")
        scp = sc_cm.__enter__()
        atp_cm = tc.tile_pool(name="at_psum", bufs=at_bufs, space="PSUM")
        atp = atp_cm.__enter__()
        dn_cm = tc.tile_pool(name="dn_psum", bufs=1, space="PSUM")
        dnp = dn_cm.__enter__()
        qk_cm = tc.tile_pool(name="qk_psum", bufs=qk_bufs, space="PSUM")
        qkp = qk_cm.__enter__()

        def qk_mm(t, sb, k, box):
            if k == 0:
                box["ps"] = qkp.tile([128, 512], F32, tag="qk",
                                     name=f"qkps_t{t}_sb{sb}")
            nc.tensor.matmul(
                box["ps"], wqk_sb[:, k, t * DH:(t + 1) * DH],
                xT_sb[:, k, sb * 512:(sb + 1) * 512],
                start=(k == 0), stop=(k == KT - 1),
            )

        def qk_copy(t, sb, box):
            h = t // 2
            is_q = t % 2 == 0
            dst = (qT_sb if is_q else kT_sb)[:, h, sb * 512:(sb + 1) * 512]
            if is_q:
                nc.scalar.activation(dst, box["ps"], Ident, bias=bq_sb[:, h])
            else:
                nc.scalar.activation(dst, box["ps"], Copy)

        for t in range(NQK - 1):
            for sb in range(SB):
                box = {}
                for k in range(KT):
                    qk_mm(t, sb, k, box)
                qk_copy(t, sb, box)

        t_def = NQK - 1
        filler = deque()
        for sb in range(SB):
            box = {}
            for k in range(KT):
                filler.append(lambda t=t_def, sb=sb, k=k, box=box:
                              qk_mm(t, sb, k, box))
            filler.append(lambda t=t_def, sb=sb, box=box: qk_copy(t, sb, box))

        # ---- window 2: attention (sb-major) + interleaved out-projection
        atsb_cm = tc.tile_pool(name="at_sb", bufs=at_sb_bufs)
        atsb = atsb_cm.__enter__()
        u_cm = tc.tile_pool(name="u_pool", bufs=u_bufs)
        up = u_cm.__enter__()
        usum_cm = tc.tile_pool(name="usum_pool", bufs=usum_bufs)
        usp = usum_cm.__enter__()
        rcp_cm = tc.tile_pool(name="recip_pool", bufs=2)
        rcp = rcp_cm.__enter__()
        rb_cm = tc.tile_pool(name="rb_pool", bufs=rb_bufs)
        rbp = rb_cm.__enter__()
        stage_cm = tc.tile_pool(name="stage_pool", bufs=stage_bufs)
        stp = stage_cm.__enter__()
        op_pool_box = {}

        def emit_att_block(h, sb, at_tile, pops, pops_per_slot,
                           delay_pops=False, prev_epilogue=None):
            """One attention block: scores, exp, attn@V.  `pops` is a deque
            of deferred PE work (out-proj chunks / deferred qk projection)
            interleaved to keep PE ahead of Act.  Returns the epilogue
            closure (softmax denominator matmul + reciprocal + broadcast +
            normalize) — the caller emits it early in the NEXT block so the
            tensor engine never waits on the DVE usum stream."""
            sq = slice(sb * 512, (sb + 1) * 512)
            at_ps = atp.tile([128, 512], F32, tag="at", name=f"atps_{h}_{sb}")
            usum = usp.tile([128, 512], DT, tag="usum")
            us = {}

            def pop(nmax):
                c = 0
                while pops and c < nmax:
                    pops.popleft()()
                    c += 1

            def emit_sc(sk):
                sc_t = scp.tile([128, 512], F32, tag="sc",
                                name=f"sc_{h}_{sb}_{sk}")
                nc.tensor.matmul(
                    sc_t, kT_sb[:, h, sk * 128:(sk + 1) * 128],
                    qT_sb[:, h, sq], start=True, stop=True,
                )
                u = up.tile([128, 512], DT, tag="u", name=f"u_{h}_{sb}_{sk}")
                nc.scalar.activation(u, sc_t, Exp)
                us[sk] = u
                if sk == 0:
                    nc.vector.tensor_copy(out=usum, in_=u)
                else:
                    nc.vector.tensor_tensor(
                        out=usum, in0=usum, in1=u, op=mybir.AluOpType.add)

            def emit_av(sk):
                nc.tensor.matmul(
                    at_ps, v_sb[:, sk, h * DH:(h + 1) * DH], us[sk],
                    start=(sk == 0), stop=(sk == ST - 1),
                )

            emit_sc(0)
            emit_sc(1)
            emit_sc(2)
            if prev_epilogue is not None:
                prev_epilogue()
            if not delay_pops:
                pop(pops_per_slot)
            for sk in range(ST):
                if pops_per_slot > 1 or (sk % 4 == 3 and
                                         not (delay_pops and sk < 5)):
                    pop(pops_per_slot)
                if sk + 3 < ST:
                    emit_sc(sk + 3)
                emit_av(sk)

            def epilogue():
                # den and rb share one rotating PSUM bank (the [1,512] den
                # only uses the first partition row of it)
                dn1 = dnp.tile([128, 512], F32, tag="dnrb",
                               name=f"den_{h}_{sb}")
                nc.tensor.matmul(dn1[0:1, :], ones_sb, usum,
                                 start=True, stop=True)
                recip = rcp.tile([1, 512], DT, tag="recip",
                                 name=f"rcp_{h}_{sb}")
                with nc.allow_low_precision(
                        reason="fp16 reciprocal: 5e-4 rel, fp16 matmul rb"):
                    nc.vector.reciprocal(out=recip, in_=dn1[0:1, :])
                dn2 = dnp.tile([128, 512], F32, tag="dnrb",
                               name=f"rb_{h}_{sb}")
                nc.tensor.matmul(dn2, ones_row, recip, start=True, stop=True)
                rbs = rbp.tile([128, 512], F32, tag="rb",
                               name=f"rbs_{h}_{sb}")
                nc.scalar.activation(rbs, dn2, Copy)
                nc.vector.tensor_tensor(
                    out=at_tile[:, h], in0=at_ps, in1=rbs,
                    op=mybir.AluOpType.mult,
                )
            return epilogue

        def make_op_units(sb, at_tile):
            """Out-projection for sequence block sb: STB stages of EB chunks;
            each chunk = hpc accumulating matmuls + a DVE stage copy; the
            stage is DMA'd out once complete."""
            units = []
            for st_i in range(STB):
                st = sb * STB + st_i
                stage_box = {}
                for eb in range(EB):
                    def chunk(st=st, st_i=st_i, eb=eb, stage_box=stage_box,
                              at_tile=at_tile):
                        if eb == 0:
                            stage_box["t"] = stp.tile(
                                [128, e], DT, tag="stage", name=f"stage{st}")
                        ps = op_pool_box["pool"].tile(
                            [128, 512], F32, tag="op", name=f"op{st}_{eb}")
                        for h in range(hpc):
                            nc.tensor.matmul(
                                ps, at_tile[:, h, st_i * 128:(st_i + 1) * 128],
                                wo_sb[:, h, eb * 512:(eb + 1) * 512],
                                start=(h == 0), stop=(h == hpc - 1),
                            )
                        stage = stage_box["t"]
                        sl = slice(eb * 512, (eb + 1) * 512)
                        # split stage copies between Act and DVE: PE
                        # consumers wait on DVE semaphore counts, so bulk
                        # copies queued on DVE delay every later normalize
                        # in its stream — give half to Act (GPSIMD cannot
                        # read PSUM per the walrus verifier).
                        if st == ST - 1 and eb == EB - 1:
                            # final chunk: copy halves on both engines in
                            # parallel, quarter DMAs — shortest tail
                            nc.scalar.activation(
                                stage[:, eb * 512:eb * 512 + 256],
                                ps[:, 0:256], Copy)
                            nc.vector.tensor_copy(
                                out=stage[:, eb * 512 + 256:(eb + 1) * 512],
                                in_=ps[:, 256:512])
                            nc.sync.dma_start(
                                out=out[st * 128:(st + 1) * 128,
                                        eb * 512:eb * 512 + 256],
                                in_=stage[:, eb * 512:eb * 512 + 256])
                            nc.sync.dma_start(
                                out=out[st * 128:(st + 1) * 128,
                                        eb * 512 + 256:(eb + 1) * 512],
                                in_=stage[:, eb * 512 + 256:(eb + 1) * 512])
                            return
                        if eb % 2 == 0:
                            nc.scalar.activation(stage[:, sl], ps, Copy)
                        else:
                            nc.vector.tensor_copy(out=stage[:, sl], in_=ps)
                        if st == ST - 1:
                            # last tile: DMA per chunk so the final
                            # writeback latency is short
                            nc.sync.dma_start(
                                out=out[st * 128:(st + 1) * 128, sl],
                                in_=stage[:, sl])
                        elif eb == EB // 2 - 1:
                            nc.sync.dma_start(
                                out=out[st * 128:(st + 1) * 128, :e // 2],
                                in_=stage[:, :e // 2])
                        elif eb == EB - 1:
                            nc.sync.dma_start(
                                out=out[st * 128:(st + 1) * 128, e // 2:],
                                in_=stage[:, e // 2:])
                    units.append(chunk)
            return units

        op_queue = deque()
        at_tiles = {}
        flush_runway = []
        epi = None
        for sb in range(SB):
            if sb == SB - 1 and len(op_queue) >= 5:
                # hold back a few fully-ready chunks of the previous block:
                # they give PE runway at the final flush while the last
                # softmax chain drains
                flush_runway = [op_queue.pop() for _ in range(5)][::-1]
            at_t = atsb.tile([128, hpc, 512], DT, tag="at", name=f"at{sb}")
            at_tiles[sb] = at_t
            for h in range(hpc):
                if sb == 0:
                    if h == hpc - 1:
                        while filler:  # t_def copy must precede h3's scores
                            filler.popleft()()
                        epi = emit_att_block(h, sb, at_t, op_queue, 0,
                                             prev_epilogue=epi)
                    else:
                        epi = emit_att_block(
                            h, sb, at_t, filler, 3,
                            prev_epilogue=epi if h else None)
                else:
                    # the first block after a cycle boundary must not pop an
                    # out-proj chunk immediately: the previous cycle's last
                    # normalize is still in flight
                    epi = emit_att_block(h, sb, at_t, op_queue, 1,
                                         delay_pops=(h == 0),
                                         prev_epilogue=epi)
            if sb == 0:
                qk_cm.__exit__(None, None, None)
                op_cm = tc.tile_pool(name="op_psum", bufs=op_bufs,
                                     space="PSUM")
                op_pool_box["pool"] = op_cm.__enter__()
            op_queue.extend(make_op_units(sb, at_t))
        # final flush: runway chunks first, the last block's epilogue woven
        # between them, then the rest of the out-projection
        for i, u in enumerate(flush_runway):
            u()
            if i == 1:
                epi()
                epi = None
        if epi is not None:
            epi()
        while op_queue:
            op_queue.popleft()()

        op_cm.__exit__(None, None, None)
        dn_cm.__exit__(None, None, None)
        atp_cm.__exit__(None, None, None)
        sc_cm.__exit__(None, None, None)
        stage_cm.__exit__(None, None, None)
        rb_cm.__exit__(None, None, None)
        rcp_cm.__exit__(None, None, None)
        usum_cm.__exit__(None, None, None)
        u_cm.__exit__(None, None, None)
        atsb_cm.__exit__(None, None, None)
        projx_cm.__exit__(None, None, None)
        singles_cm.__exit__(None, None, None)
    if split_waits:
        _split_waits(nc)
    return nc


# ---------------------------------------------------------------------------
# host-side sharding / gather


def _np_bf16(a):
    return np.ascontiguousarray(a.astype(ml_dtypes.bfloat16))


def make_in_maps(x, W_qkv, b_qkv, W_out, mm_np_dtype=np.float16):
    """Shard full inputs into 8 per-core input dicts."""
    x = np.asarray(x, dtype=np.float32)
    W_qkv = np.asarray(W_qkv, dtype=np.float32)
    b_qkv = np.asarray(b_qkv, dtype=np.float32)
    W_out = np.asarray(W_out, dtype=np.float32)
    scale = 1.0 / np.sqrt(DH)

    def cast(a):
        return np.ascontiguousarray(a.astype(mm_np_dtype))

    in_maps = []
    for c in range(NCORES):
        b = c // (NCORES // B)
        g = c % (NCORES // B)
        heads = range(g * HPC, (g + 1) * HPC)
        wqk_cols = []
        bq_rows = []
        wv_cols = []
        for hh in heads:
            base = hh * 3 * DH
            wqk_cols.append(W_qkv[:, base:base + DH] * scale)        # q
            wqk_cols.append(W_qkv[:, base + DH:base + 2 * DH])       # k
            bq_rows.append(b_qkv[base:base + DH] * scale)
            wv_cols.append(W_qkv[:, base + 2 * DH:base + 3 * DH])    # v
        in_maps.append({
            "xT": cast(x[b].T),
            "wqk": cast(np.concatenate(wqk_cols, axis=1)),
            "bq": np.ascontiguousarray(
                np.stack(bq_rows)[:, :, None], dtype=np.float32),
            "wv": cast(np.concatenate(wv_cols, axis=1)),
            "wo": cast(W_out[g * HPC * DH:(g + 1) * HPC * DH, :]),
        })
    return in_maps


def gather_output(results, b_qkv, W_out, b_out):
    """Sum per-core partials and add the folded biases."""
    b_qkv = np.asarray(b_qkv, dtype=np.float32)
    W_out = np.asarray(W_out, dtype=np.float32)
    b_out = np.asarray(b_out, dtype=np.float32)
    bv_cat = np.concatenate(
        [b_qkv[hh * 3 * DH + 2 * DH: hh * 3 * DH + 3 * DH] for hh in range(H)]
    )
    bias_eff = bv_cat @ W_out + b_out
    gpb = NCORES // B
    out = np.empty((B, S, E), dtype=np.float32)
    for b in range(B):
        acc = results[b * gpb]["out"].astype(np.float32)
        for g in range(1, gpb):
            acc += results[b * gpb + g]["out"].astype(np.float32)
        out[b] = acc + bias_eff
    return out


# ---------------------------------------------------------------------------
# public entry point

_CACHE = {}


def kernel(x, W_qkv, b_qkv, W_out, b_out):
    from concourse.bass_utils import run_bass_kernel_spmd

    _install_neff_cache()
    if "nc" not in _CACHE:
        _CACHE["nc"] = build_nc()
    nc = _CACHE["nc"]
    in_maps = make_in_maps(x, W_qkv, b_qkv, W_out)
    res = run_bass_kernel_spmd(nc, in_maps, core_ids=list(range(NCORES)))
    return gather_output(res.results, b_qkv, W_out, b_out)


# revision 53
# speedup vs baseline: 1.0185x; 1.0185x over previous
"""Multi-head attention (B=2, S=2048, E=2048, H=16) on 8 Trainium2 NeuronCores.

Sharding: core c = 4*b + g handles batch b and head group g (4 heads).
Each core computes QKV projections for its heads, attention, and its partial
output projection; the host sums the 4 partials per batch and adds the
(folded) biases.

Bias folding (host side):
  - q bias: applied on-chip (per-partition bias in the qT projection copy).
  - k bias: dropped — adds a per-query constant to scores, softmax-invariant.
  - v bias: since softmax weights sum to 1, it shifts attn by bv exactly;
    folded into the final bias as bv_cat @ W_out.
  - out bias: added on host after summing partials.

Schedule (single pass, engines overlapped):
  window 1: V projection pipelined against the serial DMA pipe (groups of 8
    sequence tiles accumulate in 8 PSUM banks while xT/wv k-tiles stream in),
    then Q/K projection for all but the last tile; the last q/k tile is
    deferred into the first attention cycle as PE filler.
  window 2: attention emitted sb-major (4 heads per sb block); out-projection
    chunks of the previous sb are interleaved between attention matmuls so
    the Act engine's exp stream (the secondary bottleneck) hides under PE
    work.  Softmax denominator = DVE usum adds + fp16 ones-matmul; the
    epilogue (den/recip/broadcast/normalize) is deferred into the next
    block so the tensor engine never waits on the DVE stream.
"""

import os
import shutil
import hashlib
from collections import deque

import numpy as np
import ml_dtypes

import concourse.bass as bass
import concourse.mybir as mybir
import concourse.tile as tile

# ---------------------------------------------------------------------------
# problem constants (hardcoded per contest rules)
B, S, E, H, DH = 2, 2048, 2048, 16, 128
NCORES = 8
HPC = H // (NCORES // B)  # heads per core = 4

F32 = mybir.dt.float32


_GATE_SEM_ID = 170  # Tile uses ids ~151-165; this one is free


def _split_waits(nc, limit=1):
    """Cap sync-waits per instruction (walrus here rejects more than one).

    Compute-engine instructions block their sequencer anyway, so surplus
    waits move onto same-engine nops inserted immediately before them —
    semantics unchanged.

    DMA instructions are dispatched asynchronously to DGE queues; making
    the issuing sequencer block on their waits can deadlock. Instead, a
    chain of Pool-engine nops takes over ALL of the DMA's waits and finally
    increments a dedicated gate semaphore; the DMA waits only on the gate.
    The gate semaphore is returned to zero at the very end so repeated NEFF
    executions see a clean state.
    """
    import bass_rust

    counter = [0]
    gate_count = [0]

    def mk_nop(engine, hint, waits, updates):
        counter[0] += 1
        nop = mybir.InstNoOp(
            name=f"W-split-{counter[0]}", opcode="NoOp",
            engine=engine, ins=[], outs=[],
            text_hint=hint, bass_nofuse=True,
        )
        nop.sync_info = bass_rust.SyncInfo(on_wait=waits, on_update=updates)
        nc.register_instruction(nop, overwrite=True)
        return nop

    last_blk = None
    for fn in nc.m.functions:
        for blk in fn.blocks:
            old = list(blk.instructions)
            if old:
                last_blk = blk
            new = []
            changed = False
            for inst in old:
                si = inst.sync_info
                n = len(si.on_wait) if si is not None else 0
                if n > limit:
                    waits = list(si.on_wait)
                    changed = True
                    if "DMA" in type(inst).__name__:
                        gate_count[0] += 1
                        for j, w in enumerate(waits):
                            upd = []
                            if j == len(waits) - 1:
                                upd = [bass_rust.SyncUpdate(
                                    sync_type="semaphore", id=_GATE_SEM_ID,
                                    update_mode="sem-inc", update_value=1,
                                    ant_name="dma_gate")]
                            new.append(mk_nop(
                                mybir.EngineType.Pool, "dma_gate", [w], upd))
                        si.on_wait = [bass_rust.SyncWait(
                            sync_type="semaphore", id=_GATE_SEM_ID,
                            wait_mode="sem-ge-imm",
                            wait_value=gate_count[0], ant_name="dma_gate")]
                        inst.sync_info = si
                    else:
                        for j in range(limit, n, limit):
                            new.append(mk_nop(
                                inst.engine, "wait_split",
                                waits[j:j + limit], []))
                        si.on_wait = waits[:limit]
                        inst.sync_info = si
                new.append(inst)
            if changed:
                blk.instructions = new

    if gate_count[0] and last_blk is not None:
        reset = mk_nop(mybir.EngineType.Pool, "gate_reset", [], [
            bass_rust.SyncUpdate(
                sync_type="semaphore", id=_GATE_SEM_ID,
                update_mode="sem-sub-imm", update_value=gate_count[0],
                ant_name="dma_gate")])
        lst = list(last_blk.instructions)
        lst.append(reset)
        last_blk.instructions = lst


def _install_neff_cache():
    """Persistently cache walrus-compiled NEFFs keyed on the BIR hash."""
    import concourse.bass2jax as b2j
    import concourse.bass_utils as bu

    if getattr(b2j, "_ant_neff_cache_installed", False):
        return
    cache_dir = os.path.expanduser("~/.cache/bass_neff_cache")
    os.makedirs(cache_dir, exist_ok=True)
    orig = bu.compile_bir_kernel

    def cached_compile(bir_json, tmpdir, neff_name="file.neff"):
        if isinstance(bir_json, str):
            bir_bytes = bir_json.encode()
        else:
            bir_bytes = bir_json
        h = hashlib.sha256(bir_bytes).hexdigest()
        path = os.path.join(cache_dir, f"{h}_{neff_name}")
        if os.path.exists(path):
            os.makedirs(os.path.join(tmpdir, "sg00"), exist_ok=True)
            out = os.path.join(tmpdir, "sg00", neff_name)
            shutil.copy(path, out)
            return out
        out = orig(bir_json, tmpdir, neff_name)
        shutil.copy(out, path)
        return out

    b2j.compile_bir_kernel = cached_compile
    bu.compile_bir_kernel = cached_compile
    b2j._ant_neff_cache_installed = True


# ---------------------------------------------------------------------------
# kernel builder (dims parameterized so a downsized version can run in CoreSim)


def build_nc(s=S, e=E, hpc=HPC, mm_dtype=mybir.dt.float16, split_waits=True,
             vgroup=8, qk_bufs=2, sc_bufs=2, at_bufs=2, op_bufs=2,
             u_bufs=5, usum_bufs=2, rb_bufs=2, stage_bufs=3,
             at_sb_bufs=2):
    """Build the per-core Bass program.

    DRAM inputs (per core):
      xT  : (e, s)        x[b].T
      wqk : (e, 2*hpc*DH) [q_h0 | k_h0 | q_h1 | k_h1 | ...], q pre-scaled
      bq  : (hpc, DH, 1)  q biases (pre-scaled)
      wv  : (e, hpc*DH)   [v_h0 | v_h1 | ...]
      wo  : (hpc*DH, e)   W_out rows for this core's heads
    DRAM output:
      out : (s, e) fp16   partial output (no biases)
    """
    KT = e // 128   # contraction tiles
    ST = s // 128   # sequence tiles
    SB = s // 512   # sequence blocks
    EB = e // 512   # embedding blocks
    STB = ST // SB  # sequence tiles per block (4)
    NQK = 2 * hpc   # q/k projection output tiles
    DT = mm_dtype
    Exp = mybir.ActivationFunctionType.Exp
    Copy = mybir.ActivationFunctionType.Copy
    Ident = mybir.ActivationFunctionType.Identity

    nc = bass.Bass("TRN2", target_bir_lowering=False, debug=False,
                   num_devices=NCORES)
    xT = nc.dram_tensor("xT", [e, s], DT, kind="ExternalInput").ap()
    wqk = nc.dram_tensor("wqk", [e, NQK * DH], DT, kind="ExternalInput").ap()
    bq = nc.dram_tensor("bq", [hpc, DH, 1], F32, kind="ExternalInput").ap()
    wv = nc.dram_tensor("wv", [e, hpc * DH], DT, kind="ExternalInput").ap()
    wo = nc.dram_tensor("wo", [hpc * DH, e], DT, kind="ExternalInput").ap()
    out = nc.dram_tensor("out", [s, e], DT, kind="ExternalOutput").ap()

    with tile.TileContext(nc) as tc:
        singles_cm = tc.tile_pool(name="singles", bufs=1)
        singles = singles_cm.__enter__()
        projx_cm = tc.tile_pool(name="projx", bufs=1)
        projx = projx_cm.__enter__()
        wv_cm = tc.tile_pool(name="wv_pool", bufs=1)
        wvp = wv_cm.__enter__()

        # ---- SBUF tensors
        xT_sb = projx.tile([128, KT, s], DT, tag="xT")
        wqk_sb = projx.tile([128, KT, NQK * DH], DT, tag="wqk")
        wv_sb = wvp.tile([128, KT, hpc * DH], DT, tag="wv")
        wo_sb = singles.tile([128, hpc, e], DT, tag="wo")
        bq_sb = singles.tile([128, hpc, 1], F32, tag="bq")
        qT_sb = singles.tile([128, hpc, s], DT, tag="qT")
        kT_sb = singles.tile([128, hpc, s], DT, tag="kT")
        v_sb = singles.tile([128, ST, hpc * DH], DT, tag="v")
        ones_sb = singles.tile([128, 1], DT, tag="ones")
        ones_row = singles.tile([1, 128], DT, tag="ones_row")
        nc.vector.memset(ones_sb, 1.0)
        nc.vector.memset(ones_row, 1.0)

        # ---- input DMAs: issue order == the serial DMA pipe's service order.
        # v-projection consumes (wv[k], xT[k]) pairs first; wqk afterwards.
        xT_r = xT.rearrange("(ko ki) q -> ki ko q", ki=128)
        wqk_r = wqk.rearrange("(ko ki) q -> ki ko q", ki=128)
        wv_r = wv.rearrange("(ko ki) q -> ki ko q", ki=128)
        wo_r = wo.rearrange("(ho hi) q -> hi ho q", hi=128)
        for k in range(KT):
            if k == 0:
                # xT's first half lands first (the PE warmups below need
                # only it), split so the first real matmul starts sooner
                half = s // 2
                nc.sync.dma_start(out=xT_sb[:, 0, :half],
                                  in_=xT_r[:, 0, :half])
                nc.sync.dma_start(out=wv_sb[:, 0], in_=wv_r[:, 0])
                nc.sync.dma_start(out=xT_sb[:, 0, half:],
                                  in_=xT_r[:, 0, half:])
            else:
                nc.sync.dma_start(out=wv_sb[:, k], in_=wv_r[:, k])
                nc.sync.dma_start(out=xT_sb[:, k], in_=xT_r[:, k])
        for k in range(KT):
            nc.sync.dma_start(out=wqk_sb[:, k], in_=wqk_r[:, k])
        for h in range(hpc):
            nc.sync.dma_start(out=wo_sb[:, h], in_=wo_r[:, h])
            nc.sync.dma_start(out=bq_sb[:, h], in_=bq[h])

        # ---- window 1a: V projection.  The first group runs k-outer so it
        # consumes each (wv[k], xT[k]) pair right as it lands from the DMA
        # pipe.  Later groups run st-outer (inputs are resident by then) so
        # the PSUM drain copies are staggered across the whole group — the
        # next pool's first use waits on ALL of this pool's reads (Tile
        # tracks write-after-read at pool granularity), so the last copy
        # must land right after the last matmul, not 8 serial copies later.
        with tc.tile_pool(name="v_psum", bufs=vgroup, space="PSUM") as vp:
            for g0 in range(0, ST, vgroup):
                n = min(vgroup, ST - g0)
                ps = [vp.tile([128, hpc * DH], F32, tag="v",
                              name=f"vps{g0}_{i}") for i in range(n)]
                if g0 == 0:
                    # tiny warmup matmuls absorb the tensor engine's p-state
                    # ramp while the first DMA lands (they share its gate)
                    for w in range(40):
                        nc.tensor.matmul(
                            ps[0][:32, :32], xT_sb[:, 0, 0:32],
                            xT_sb[:, 0, 0:32], start=True, stop=True,
                            skip_group_check=True,
                        )
                    for k in range(KT):
                        for i in range(n):
                            st = g0 + i
                            nc.tensor.matmul(
                                ps[i], xT_sb[:, k, st * 128:(st + 1) * 128],
                                wv_sb[:, k], start=(k == 0),
                                stop=(k == KT - 1),
                            )
                            if k == KT - 1:
                                nc.scalar.activation(
                                    v_sb[:, g0 + i], ps[i], Copy)
                else:
                    for i in range(n):
                        st = g0 + i
                        for k in range(KT):
                            nc.tensor.matmul(
                                ps[i], xT_sb[:, k, st * 128:(st + 1) * 128],
                                wv_sb[:, k], start=(k == 0),
                                stop=(k == KT - 1),
                            )
                        nc.scalar.activation(v_sb[:, g0 + i], ps[i], Copy)
        wv_cm.__exit__(None, None, None)

        # ---- window 1b: Q/K projections; the last tile (k of the last head)
        # is deferred into the first attention cycle as PE filler.
        # PSUM pool stack (LIFO): sc, atp outlive qk; op replaces qk after
        # the first attention cycle.  2*2 + 2 + 2 = 8 banks.
        sc_cm = tc.tile_pool(name="sc_psum", bufs=sc_bufs, space="PSUM")
        scp = sc_cm.__enter__()
        atp_cm = tc.tile_pool(name="at_psum", bufs=at_bufs, space="PSUM")
        atp = atp_cm.__enter__()
        qk_cm = tc.tile_pool(name="qk_psum", bufs=qk_bufs, space="PSUM")
        qkp = qk_cm.__enter__()

        def qk_mm(t, sb, k, box):
            if k == 0:
                box["ps"] = qkp.tile([128, 512], F32, tag="qk",
                                     name=f"qkps_t{t}_sb{sb}")
            nc.tensor.matmul(
                box["ps"], wqk_sb[:, k, t * DH:(t + 1) * DH],
                xT_sb[:, k, sb * 512:(sb + 1) * 512],
                start=(k == 0), stop=(k == KT - 1),
            )

        def qk_copy(t, sb, box):
            h = t // 2
            is_q = t % 2 == 0
            dst = (qT_sb if is_q else kT_sb)[:, h, sb * 512:(sb + 1) * 512]
            if is_q:
                nc.scalar.activation(dst, box["ps"], Ident, bias=bq_sb[:, h])
            else:
                nc.scalar.activation(dst, box["ps"], Copy)

        for t in range(NQK - 1):
            for sb in range(SB):
                box = {}
                for k in range(KT):
                    qk_mm(t, sb, k, box)
                qk_copy(t, sb, box)

        t_def = NQK - 1
        filler = deque()
        for sb in range(SB):
            box = {}
            for k in range(KT):
                filler.append(lambda t=t_def, sb=sb, k=k, box=box:
                              qk_mm(t, sb, k, box))
            filler.append(lambda t=t_def, sb=sb, box=box: qk_copy(t, sb, box))

        # ---- window 2: attention (sb-major) + interleaved out-projection
        atsb_cm = tc.tile_pool(name="at_sb", bufs=at_sb_bufs)
        atsb = atsb_cm.__enter__()
        u_cm = tc.tile_pool(name="u_pool", bufs=u_bufs)
        up = u_cm.__enter__()
        usum_cm = tc.tile_pool(name="usum_pool", bufs=usum_bufs)
        usp = usum_cm.__enter__()
        rcp_cm = tc.tile_pool(name="recip_pool", bufs=2)
        rcp = rcp_cm.__enter__()
        rb_cm = tc.tile_pool(name="rb_pool", bufs=rb_bufs)
        rbp = rb_cm.__enter__()
        stage_cm = tc.tile_pool(name="stage_pool", bufs=stage_bufs)
        stp = stage_cm.__enter__()
        op_pool_box = {}

        def emit_att_block(h, sb, at_tile, pops, pops_per_slot,
                           delay_pops=False, prev_epilogue=None):
            """One attention block: scores, exp, attn@V.  `pops` is a deque
            of deferred PE work (out-proj chunks / deferred qk projection)
            interleaved to keep PE ahead of Act.  Returns the epilogue
            closure (softmax denominator matmul + reciprocal + broadcast +
            normalize) — the caller emits it early in the NEXT block so the
            tensor engine never waits on the DVE usum stream."""
            sq = slice(sb * 512, (sb + 1) * 512)
            at_ps = atp.tile([128, 512], F32, tag="at", name=f"atps_{h}_{sb}")
            usum = usp.tile([128, 512], DT, tag="usum")
            us = {}

            def pop(nmax):
                c = 0
                while pops and c < nmax:
                    pops.popleft()()
                    c += 1

            npairs = ST // 2

            def emit_pair(p):
                sc_t = scp.tile([128, 1024], F32, tag="sc",
                                name=f"sc_{h}_{sb}_{p}")
                for j in (0, 1):
                    sk = 2 * p + j
                    nc.tensor.matmul(
                        sc_t[:, j * 512:(j + 1) * 512],
                        kT_sb[:, h, sk * 128:(sk + 1) * 128],
                        qT_sb[:, h, sq], start=True, stop=True,
                    )
                u = up.tile([128, 1024], DT, tag="u", name=f"u_{h}_{sb}_{p}")
                nc.scalar.activation(u, sc_t, Exp)
                us[2 * p] = u[:, 0:512]
                us[2 * p + 1] = u[:, 512:1024]
                if p == 0:
                    nc.vector.tensor_copy(out=usum, in_=u[:, 0:512])
                nc.vector.tensor_tensor(
                    out=usum, in0=usum,
                    in1=u[:, 0:512] if p else u[:, 512:1024],
                    op=mybir.AluOpType.add)
                if p:
                    nc.vector.tensor_tensor(
                        out=usum, in0=usum, in1=u[:, 512:1024],
                        op=mybir.AluOpType.add)

            def emit_av(sk):
                nc.tensor.matmul(
                    at_ps, v_sb[:, sk, h * DH:(h + 1) * DH], us[sk],
                    start=(sk == 0), stop=(sk == ST - 1),
                )

            emit_pair(0)
            emit_pair(1)
            if prev_epilogue is not None:
                prev_epilogue()
            if not delay_pops:
                pop(pops_per_slot)
            for p in range(npairs):
                if pops_per_slot > 1 or (p % 2 == 1 and
                                         not (delay_pops and p < 3)):
                    pop(pops_per_slot)
                if p + 2 < npairs:
                    emit_pair(p + 2)
                emit_av(2 * p)
                emit_av(2 * p + 1)

            def epilogue():
                # den and rb live in a score-pool tile: den uses the first
                # partition row of the left half, rb the right half — no
                # dedicated PSUM bank needed
                dn = scp.tile([128, 1024], F32, tag="sc", name=f"dn_{h}_{sb}")
                nc.tensor.matmul(dn[0:1, 0:512], ones_sb, usum,
                                 start=True, stop=True)
                recip = rcp.tile([1, 512], DT, tag="recip",
                                 name=f"rcp_{h}_{sb}")
                with nc.allow_low_precision(
                        reason="fp16 reciprocal: 5e-4 rel, fp16 matmul rb"):
                    nc.vector.reciprocal(out=recip, in_=dn[0:1, 0:512])
                nc.tensor.matmul(dn[:, 512:1024], ones_row, recip,
                                 start=True, stop=True)
                rbs = rbp.tile([128, 512], F32, tag="rb",
                               name=f"rbs_{h}_{sb}")
                nc.scalar.activation(rbs, dn[:, 512:1024], Copy)
                nc.vector.tensor_tensor(
                    out=at_tile[:, h], in0=at_ps, in1=rbs,
                    op=mybir.AluOpType.mult,
                )
            return epilogue

        def make_op_units(sb, at_tile):
            """Out-projection for sequence block sb: STB stages of EB chunks;
            each chunk = hpc accumulating matmuls + a DVE stage copy; the
            stage is DMA'd out once complete."""
            units = []
            for st_i in range(STB):
                st = sb * STB + st_i
                stage_box = {}
                for eb in range(EB):
                    def chunk(st=st, st_i=st_i, eb=eb, stage_box=stage_box,
                              at_tile=at_tile):
                        if eb == 0:
                            stage_box["t"] = stp.tile(
                                [128, e], DT, tag="stage", name=f"stage{st}")
                        ps = op_pool_box["pool"].tile(
                            [128, 512], F32, tag="op", name=f"op{st}_{eb}")
                        for h in range(hpc):
                            nc.tensor.matmul(
                                ps, at_tile[:, h, st_i * 128:(st_i + 1) * 128],
                                wo_sb[:, h, eb * 512:(eb + 1) * 512],
                                start=(h == 0), stop=(h == hpc - 1),
                            )
                        stage = stage_box["t"]
                        sl = slice(eb * 512, (eb + 1) * 512)
                        # split stage copies between Act and DVE: PE
                        # consumers wait on DVE semaphore counts, so bulk
                        # copies queued on DVE delay every later normalize
                        # in its stream — give half to Act (GPSIMD cannot
                        # read PSUM per the walrus verifier).
                        if st == ST - 1 and eb == EB - 1:
                            # final chunk: copy halves on both engines in
                            # parallel, quarter DMAs — shortest tail
                            nc.scalar.activation(
                                stage[:, eb * 512:eb * 512 + 256],
                                ps[:, 0:256], Copy)
                            nc.vector.tensor_copy(
                                out=stage[:, eb * 512 + 256:(eb + 1) * 512],
                                in_=ps[:, 256:512])
                            nc.sync.dma_start(
                                out=out[st * 128:(st + 1) * 128,
                                        eb * 512:eb * 512 + 256],
                                in_=stage[:, eb * 512:eb * 512 + 256])
                            nc.sync.dma_start(
                                out=out[st * 128:(st + 1) * 128,
                                        eb * 512 + 256:(eb + 1) * 512],
                                in_=stage[:, eb * 512 + 256:(eb + 1) * 512])
                            return
                        if eb % 2 == 0:
                            nc.scalar.activation(stage[:, sl], ps, Copy)
                        else:
                            nc.vector.tensor_copy(out=stage[:, sl], in_=ps)
                        if st == ST - 1:
                            # last tile: DMA per chunk so the final
                            # writeback latency is short
                            nc.sync.dma_start(
                                out=out[st * 128:(st + 1) * 128, sl],
                                in_=stage[:, sl])
                        elif eb == EB // 2 - 1:
                            nc.sync.dma_start(
                                out=out[st * 128:(st + 1) * 128, :e // 2],
                                in_=stage[:, :e // 2])
                        elif eb == EB - 1:
                            nc.sync.dma_start(
                                out=out[st * 128:(st + 1) * 128, e // 2:],
                                in_=stage[:, e // 2:])
                    units.append(chunk)
            return units

        op_queue = deque()
        at_tiles = {}
        flush_runway = []
        epi = None
        for sb in range(SB):
            if sb == SB - 1 and len(op_queue) >= 5:
                # hold back a few fully-ready chunks of the previous block:
                # they give PE runway at the final flush while the last
                # softmax chain drains
                flush_runway = [op_queue.pop() for _ in range(5)][::-1]
            at_t = atsb.tile([128, hpc, 512], DT, tag="at", name=f"at{sb}")
            at_tiles[sb] = at_t
            for h in range(hpc):
                if sb == 0:
                    if h == hpc - 1:
                        while filler:  # t_def copy must precede h3's scores
                            filler.popleft()()
                        epi = emit_att_block(h, sb, at_t, op_queue, 0,
                                             prev_epilogue=epi)
                    else:
                        epi = emit_att_block(
                            h, sb, at_t, filler, 3,
                            prev_epilogue=epi if h else None)
                else:
                    # the first block after a cycle boundary must not pop an
                    # out-proj chunk immediately: the previous cycle's last
                    # normalize is still in flight
                    epi = emit_att_block(h, sb, at_t, op_queue, 1,
                                         delay_pops=(h == 0),
                                         prev_epilogue=epi)
            if sb == 0:
                qk_cm.__exit__(None, None, None)
                op_cm = tc.tile_pool(name="op_psum", bufs=op_bufs,
                                     space="PSUM")
                op_pool_box["pool"] = op_cm.__enter__()
            op_queue.extend(make_op_units(sb, at_t))
        # final flush: runway chunks first, the last block's epilogue woven
        # between them, then the rest of the out-projection
        for i, u in enumerate(flush_runway):
            u()
            if i == 1:
                epi()
                epi = None
        if epi is not None:
            epi()
        while op_queue:
            op_queue.popleft()()

        op_cm.__exit__(None, None, None)
        atp_cm.__exit__(None, None, None)
        sc_cm.__exit__(None, None, None)
        stage_cm.__exit__(None, None, None)
        rb_cm.__exit__(None, None, None)
        rcp_cm.__exit__(None, None, None)
        usum_cm.__exit__(None, None, None)
        u_cm.__exit__(None, None, None)
        atsb_cm.__exit__(None, None, None)
        projx_cm.__exit__(None, None, None)
        singles_cm.__exit__(None, None, None)
    if split_waits:
        _split_waits(nc)
    return nc


# ---------------------------------------------------------------------------
# host-side sharding / gather


def _np_bf16(a):
    return np.ascontiguousarray(a.astype(ml_dtypes.bfloat16))


def make_in_maps(x, W_qkv, b_qkv, W_out, mm_np_dtype=np.float16):
    """Shard full inputs into 8 per-core input dicts."""
    x = np.asarray(x, dtype=np.float32)
    W_qkv = np.asarray(W_qkv, dtype=np.float32)
    b_qkv = np.asarray(b_qkv, dtype=np.float32)
    W_out = np.asarray(W_out, dtype=np.float32)
    scale = 1.0 / np.sqrt(DH)

    def cast(a):
        return np.ascontiguousarray(a.astype(mm_np_dtype))

    in_maps = []
    for c in range(NCORES):
        b = c // (NCORES // B)
        g = c % (NCORES // B)
        heads = range(g * HPC, (g + 1) * HPC)
        wqk_cols = []
        bq_rows = []
        wv_cols = []
        for hh in heads:
            base = hh * 3 * DH
            wqk_cols.append(W_qkv[:, base:base + DH] * scale)        # q
            wqk_cols.append(W_qkv[:, base + DH:base + 2 * DH])       # k
            bq_rows.append(b_qkv[base:base + DH] * scale)
            wv_cols.append(W_qkv[:, base + 2 * DH:base + 3 * DH])    # v
        in_maps.append({
            "xT": cast(x[b].T),
            "wqk": cast(np.concatenate(wqk_cols, axis=1)),
            "bq": np.ascontiguousarray(
                np.stack(bq_rows)[:, :, None], dtype=np.float32),
            "wv": cast(np.concatenate(wv_cols, axis=1)),
            "wo": cast(W_out[g * HPC * DH:(g + 1) * HPC * DH, :]),
        })
    return in_maps


def gather_output(results, b_qkv, W_out, b_out):
    """Sum per-core partials and add the folded biases."""
    b_qkv = np.asarray(b_qkv, dtype=np.float32)
    W_out = np.asarray(W_out, dtype=np.float32)
    b_out = np.asarray(b_out, dtype=np.float32)
    bv_cat = np.concatenate(
        [b_qkv[hh * 3 * DH + 2 * DH: hh * 3 * DH + 3 * DH] for hh in range(H)]
    )
    bias_eff = bv_cat @ W_out + b_out
    gpb = NCORES // B
    out = np.empty((B, S, E), dtype=np.float32)
    for b in range(B):
        acc = results[b * gpb]["out"].astype(np.float32)
        for g in range(1, gpb):
            acc += results[b * gpb + g]["out"].astype(np.float32)
        out[b] = acc + bias_eff
    return out


# ---------------------------------------------------------------------------
# public entry point

_CACHE = {}


def kernel(x, W_qkv, b_qkv, W_out, b_out):
    from concourse.bass_utils import run_bass_kernel_spmd

    _install_neff_cache()
    if "nc" not in _CACHE:
        _CACHE["nc"] = build_nc()
    nc = _CACHE["nc"]
    in_maps = make_in_maps(x, W_qkv, b_qkv, W_out)
    res = run_bass_kernel_spmd(nc, in_maps, core_ids=list(range(NCORES)))
    return gather_output(res.results, b_qkv, W_out, b_out)
